# revision 1
# baseline (speedup 1.0000x reference)
"""MultiHeadDenseAttention on 8 Trainium2 NeuronCores.

Head-sharded tensor parallelism: each core computes 2 of 16 heads
(value projection slice, per-head MLP attention logits, softmax, S@V),
then an AllToAll exchanges head-blocks for row-blocks so each core
computes the output projection for its 512 rows with the full Wo.

Layouts (per core c, heads 2c / 2c+1):
  xT   [1024, 4096]  x.reshape(4096,1024).T         (feat on partitions)
  xc   [128, 4096]   xT rows [128c, 128c+128)       (this core's head cols)
  wv   [128, 1024]   Wv[128c:+128,:].T chunked      lhsT for value proj
  valueT [128, 4096] value cols for our heads, m on free dim
  vh[b] [128, 16*130] transposed value chunks + ones cols (fused softmax sum)
  logitsT/expT [128m, 512n] per m-chunk             (softmax w/o max: logits are O(1))
  S@V: out[65, 512] = vh_aug.T @ expT  (row 64 = sum of exp for normalization)
  A2A [8, 2, 65, 512], normalize after exchange, then out = act @ Wo.T.

All matmuls in float32r (fp32 with 11-bit RNE mantissa, 4x faster than
fp32 on the PE); inputs pre-rounded on host, fp32 accumulation in PSUM.
"""

import sys

if "/opt/trn_rl_repo" not in sys.path:
    sys.path.insert(0, "/opt/trn_rl_repo")

from contextlib import ExitStack

import numpy as np

import bass_rust
import concourse.bass as bass
import concourse.tile as tile
from concourse import masks, mybir
from concourse.bass_utils import run_bass_kernel_spmd

F32 = mybir.dt.float32
F32R = mybir.dt.float32r
AF = mybir.ActivationFunctionType

NC = 8            # cores
B = 2             # batch
N_SEQ = 2048      # seq len == max_seq_len (m)
FEAT = 1024
H = 16            # heads
D = 64            # head dim
NTOT = B * N_SEQ  # 4096 flattened rows
NBLK = 512        # n-block size
NB = NTOT // NBLK # 8 n-blocks (== A2A shards == cores)
MC = N_SEQ // 128 # 16 m-chunks per batch
CB = 130          # vh_aug per-chunk stride: 65 (h0+sum) + 65 (h1+sum)


def _split_sem_waits(nc, limit=1):
    """Walrus rejects instructions with more than ~1 sync wait; move the
    excess onto NOPs on the same engine inserted immediately before."""
    blocks = {}
    for f in nc.m.functions:
        for bb in f.blocks:
            blocks[bb.name] = bb
    for bb in blocks.values():
        i = 0
        while i < len(bb.instructions):
            inst = bb.instructions[i]
            si = inst.sync_info
            if si is not None and si.on_wait and len(si.on_wait) > limit:
                waits = list(si.on_wait)
                chunks = [waits[j : j + limit] for j in range(0, len(waits), limit)]
                si.on_wait = chunks[-1]
                engine = nc.engines[inst.engine]
                for chunk in chunks[:-1]:
                    d = engine.nop(nofuse=True, hint="wait_split")
                    dinst = d.ins if hasattr(d, "ins") else d
                    for ob in blocks.values():
                        if ob.instructions and ob.instructions[-1] is dinst:
                            ob.instructions.pop()
                            break
                    dinst.sync_info = bass_rust.SyncInfo(on_wait=chunk, on_update=[])
                    bb.instructions.insert(i, dinst)
                    i += 1
            i += 1
    return nc


def _rne12(x):
    """Round fp32 mantissa to 11 explicit bits (RNE) — the float32r format."""
    v = np.ascontiguousarray(x, dtype=np.float32).view(np.uint32).astype(np.uint64)
    half = np.uint64(0x7FF) + ((v >> np.uint64(12)) & np.uint64(1))
    out = ((v + half) & np.uint64(0xFFFFF000)).astype(np.uint32)
    return out.view(np.float32)


def _build(reps=1, phases="A"):
    nc = bass.Bass()

    xt_in = nc.dram_tensor("xt", [FEAT, NTOT], F32R, kind="ExternalInput")
    xc_in = nc.dram_tensor("xc", [128, NTOT], F32R, kind="ExternalInput")
    wv_in = nc.dram_tensor("wv", [128, FEAT], F32R, kind="ExternalInput")
    w1t_in = nc.dram_tensor("w1t", [128, D], F32R, kind="ExternalInput")
    b1_in = nc.dram_tensor("b1", [D, 1], F32, kind="ExternalInput")
    w2t_in = nc.dram_tensor("w2t", [65, N_SEQ], F32R, kind="ExternalInput")
    wot_in = nc.dram_tensor("wot", [128, NC * FEAT], F32R, kind="ExternalInput")
    sel_in = nc.dram_tensor("sel", [2, 128], F32R, kind="ExternalInput")
    out_ext = nc.dram_tensor("out", [NBLK, FEAT], F32, kind="ExternalOutput")

    with tile.TileContext(nc) as tc, ExitStack() as ctx:
        wp = ctx.enter_context(tc.tile_pool(name="wp", bufs=1))
        dram = ctx.enter_context(tc.tile_pool(name="dram", bufs=1, space="DRAM"))

        # ---- resident weights/constants -------------------------------
        wv = wp.tile([128, FEAT], F32R)
        nc.sync.dma_start(wv[:], wv_in[:])
        w1t = wp.tile([128, D], F32R)           # W1.T stacked twice (rows 0:64 / 64:128)
        nc.sync.dma_start(w1t[:], w1t_in[:])
        b1t = wp.tile([D, 1], F32)
        nc.sync.dma_start(b1t[:], b1_in[:])
        w2t = wp.tile([65, N_SEQ], F32R)        # W2.T with b2 as row 64
        nc.sync.dma_start(w2t[:], w2t_in[:])
        xc = wp.tile([128, NTOT], F32R)
        nc.sync.dma_start(xc[:], xc_in[:])
        sel = wp.tile([2, 128], F32R)
        nc.sync.dma_start(sel[:], sel_in[:])

        ident_f = wp.tile([128, 128], F32)
        masks.make_identity(nc, ident_f[:])
        onecol_f = wp.tile([128, 1], F32)
        nc.vector.memset(onecol_f[:], 1.0)
        onerow_f = wp.tile([1, NBLK], F32)
        nc.vector.memset(onerow_f[:], 1.0)
        onerow = wp.tile([1, NBLK], F32R)
        nc.vector.tensor_copy(onerow[:], onerow_f[:])
        wot = wp.tile([128, NC * FEAT], F32R)

        vh = [wp.tile([128, MC * CB], F32R, name=f"vh{b}", tag=f"vh{b}") for b in range(B)]

        for _rep in range(reps):
            a2a_send = [dram.tile([NC, 65, NBLK], F32, name=f"snd{h}_{_rep}") for h in range(2)]
            a2a_recv = [dram.tile([NC, 65, NBLK], F32, name=f"rcv{h}_{_rep}") for h in range(2)]

            with ExitStack() as c2:
                # one PSUM bank budget for the whole fused P1+P2 region:
                # psm: tag ph (1 bank, hid) + tag pv (1 bank, value/transposes)
                # psl: tag pl (2 bufs x 2 banks, double-wide logits)
                # pso: tag po (2 bufs, S@V accumulators)
                psm = c2.enter_context(tc.tile_pool(name="psm", bufs=2, space="PSUM"))
                psl = c2.enter_context(tc.tile_pool(name="psl", bufs=2, space="PSUM"))
                pso = c2.enter_context(tc.tile_pool(name="pso", bufs=2, space="PSUM"))
                hp = c2.enter_context(tc.tile_pool(name="hp", bufs=2))
                ep = c2.enter_context(tc.tile_pool(name="ep", bufs=4))
                op = c2.enter_context(tc.tile_pool(name="op", bufs=4))
                vap = c2.enter_context(tc.tile_pool(name="vap", bufs=1))

                # ---- hid MLP (only needs xc); row 64 = ones for the
                # fused b2 row in the logits matmul ----------------------
                hidTs = []
                for h in range(2):
                    hidT = hp.tile([65, NTOT], F32R, name=f"hidT{h}", tag="hidT")
                    hidTs.append(hidT)
                    for nb in range(NB):
                        ph = psm.tile([128, NBLK], F32, tag="pm", name="ph")
                        nc.tensor.matmul(
                            ph[0:D, :],
                            w1t[h * D : (h + 1) * D, :],
                            xc[h * D : (h + 1) * D, nb * NBLK : (nb + 1) * NBLK],
                            start=True,
                            stop=True,
                            skip_group_check=True,
                        )
                        nc.scalar.activation(
                            hidT[0:D, nb * NBLK : (nb + 1) * NBLK], ph[0:D, :], AF.Relu, bias=b1t[:]
                        )
                    for nb in range(NB):
                        nc.sync.dma_start(
                            hidT[D : D + 1, nb * NBLK : (nb + 1) * NBLK], onerow[:]
                        )

                # ---- P1: value projection, SBUF accumulation. Stream x
                # in quarter-column groups so vh chunks land incrementally
                # and S@V partial accumulation can start early ----------
                QW = NTOT // 4  # 1024 columns per quarter
                vacc = vap.tile([128, NTOT], F32)
                with tc.tile_pool(name="xfp", bufs=3) as xfp:
                    for q in range(4):
                        for f in range(8):
                            xf = xfp.tile([128, QW], F32R)
                            nc.sync.dma_start(
                                xf[:],
                                xt_in[f * 128 : (f + 1) * 128, q * QW : (q + 1) * QW],
                            )
                            for nbi in range(QW // NBLK):
                                nb = q * (QW // NBLK) + nbi
                                pv = psm.tile([128, NBLK], F32, tag="pm", name="pv")
                                nc.tensor.matmul(
                                    pv[:],
                                    wv[:, f * 128 : (f + 1) * 128],
                                    xf[:, nbi * NBLK : (nbi + 1) * NBLK],
                                    start=True,
                                    stop=True,
                                    skip_group_check=True,
                                )
                                dst = vacc[:, nb * NBLK : (nb + 1) * NBLK]
                                if f == 0:
                                    nc.vector.tensor_copy(dst, pv[:])
                                else:
                                    nc.vector.tensor_add(dst, dst, pv[:])
                        # transpose this quarter's m-chunks into vh
                        b = q // 2
                        for ji in range(MC // 2):
                            j = (q % 2) * (MC // 2) + ji
                            tp = psm.tile([128, NBLK], F32, tag="pm", name=f"tp{q}_{ji}")
                            nc.tensor.matmul(
                                tp[:, 0:128],
                                vacc[:, b * N_SEQ + j * 128 : b * N_SEQ + (j + 1) * 128],
                                ident_f[:],
                                is_transpose=True,
                                start=True,
                                stop=True,
                            )
                            base = j * CB
                            nc.vector.tensor_copy(vh[b][:, base : base + D], tp[:, 0:D])
                            nc.vector.tensor_copy(vh[b][:, base + 65 : base + 65 + D], tp[:, D:128])
                            nc.vector.tensor_copy(vh[b][:, base + D : base + D + 1], onecol_f[:])
                            nc.vector.tensor_copy(vh[b][:, base + 129 : base + 130], onecol_f[:])

                if _rep == 0:
                    nc.sync.dma_start(wot[:], wot_in[:])

                # ---- P2: attention ------------------------------------
                for h in range(2):
                    hidT = hidTs[h]
                    for nb in range(NB):
                        b = nb // (NB // B)
                        # quarter-size exp tiles (bufs=4): the pool slot
                        # barrier is per 4 m-chunks, not per block, so the
                        # next block's exp overlaps this block's S@V tail
                        eqs = []
                        for qt in range(4):
                            eq = ep.tile([128, 4 * NBLK], F32R, name="expTq", tag="expTq")
                            eqs.append(eq)
                            for jj in range(0, 4, 2):
                                j = qt * 4 + jj
                                pl = psl.tile([128, 2 * NBLK], F32)
                                for q in range(2):
                                    nc.tensor.matmul(
                                        pl[:, q * NBLK : (q + 1) * NBLK],
                                        w2t[:, (j + q) * 128 : (j + q + 1) * 128],
                                        hidT[:, nb * NBLK : (nb + 1) * NBLK],
                                        start=True,
                                        stop=True,
                                        skip_group_check=True,
                                    )
                                nc.scalar.activation(
                                    eq[:, jj * NBLK : (jj + 2) * NBLK], pl[:], AF.Exp
                                )
                        po = pso.tile([65, NBLK], F32)
                        for j in range(MC):
                            nc.tensor.matmul(
                                po[:],
                                vh[b][:, j * CB + h * 65 : j * CB + (h + 1) * 65],
                                eqs[j // 4][:, (j % 4) * NBLK : (j % 4 + 1) * NBLK],
                                start=(j == 0),
                                stop=(j == MC - 1),
                                skip_group_check=True,
                            )
                        ot = op.tile([65, NBLK], F32)
                        nc.vector.tensor_copy(ot[:], po[:])
                        nc.sync.dma_start(a2a_send[h][nb], ot[:])

                    # fire this head's exchange as soon as its blocks are out
                    if phases not in ("1", "2"):
                        nc.gpsimd.collective_compute(
                            "AllToAll",
                            mybir.AluOpType.bypass,
                            ins=[a2a_send[h][:].opt()],
                            outs=[a2a_recv[h][:].opt()],
                            replica_groups=[list(range(NC))],
                        )

            if phases in ("1", "2", "3"):
                continue

            # ---- P4/P5: normalize + output projection -----------------
            with ExitStack() as c4:
                rp = c4.enter_context(tc.tile_pool(name="rp", bufs=6))
                psb = c4.enter_context(tc.tile_pool(name="psb", bufs=2, space="PSUM"))
                awp = c4.enter_context(tc.tile_pool(name="awp", bufs=1))
                psw = c4.enter_context(tc.tile_pool(name="psw", bufs=3, space="PSUM"))
                obp = c4.enter_context(tc.tile_pool(name="obp", bufs=3))

                actw = [awp.tile([128, NBLK], F32R, name=f"aw{s_}", tag=f"aw{s_}") for s_ in range(NC)]
                for s in range(NC):
                    sums = rp.tile([2, NBLK], F32)
                    nc.sync.dma_start(sums[0:1, :], a2a_recv[0][s, D : D + 1, :])
                    nc.sync.dma_start(sums[1:2, :], a2a_recv[1][s, D : D + 1, :])
                    raw = rp.tile([128, NBLK], F32)
                    nc.sync.dma_start(raw[0:D, :], a2a_recv[0][s, 0:D, :])
                    nc.sync.dma_start(raw[D:128, :], a2a_recv[1][s, 0:D, :])
                    rcps_f = rp.tile([2, NBLK], F32)
                    nc.vector.reciprocal(rcps_f[:], sums[:])
                    rcps = rp.tile([2, NBLK], F32R)
                    nc.vector.tensor_copy(rcps[:], rcps_f[:])
                    pb = psb.tile([128, NBLK], F32)
                    nc.tensor.matmul(
                        pb[:], sel[:], rcps[:], start=True, stop=True,
                        skip_group_check=True,
                    )
                    nc.vector.tensor_mul(actw[s][:], raw[:], pb[:])

                for t in range(NBLK // 128):
                    p0 = psw.tile([128, 512], F32, tag="pw0")
                    p1 = psw.tile([128, 512], F32, tag="pw1")
                    for s in range(NC):
                        nc.tensor.matmul(
                            p0[:],
                            actw[s][:, t * 128 : (t + 1) * 128],
                            wot[:, s * FEAT : s * FEAT + 512],
                            start=(s == 0),
                            stop=(s == NC - 1),
                            skip_group_check=True,
                        )
                        nc.tensor.matmul(
                            p1[:],
                            actw[s][:, t * 128 : (t + 1) * 128],
                            wot[:, s * FEAT + 512 : (s + 1) * FEAT],
                            start=(s == 0),
                            stop=(s == NC - 1),
                            skip_group_check=True,
                        )
                    ob = obp.tile([128, FEAT], F32)
                    nc.vector.tensor_copy(ob[:, 0:512], p0[:])
                    nc.vector.tensor_copy(ob[:, 512:1024], p1[:])
                    nc.sync.dma_start(out_ext[t * 128 : (t + 1) * 128, :], ob[:])

    _split_sem_waits(nc)
    return nc


_CACHE = {}


def _get_program(reps=1, phases="A"):
    key = ("nc", reps, phases)
    if key not in _CACHE:
        _CACHE[key] = _build(reps, phases)
    return _CACHE[key]


def kernel(x, W1, b1, W2, b2, Wv, Wo, _run_kwargs=None):
    x = np.asarray(x, dtype=np.float32)
    W1 = np.asarray(W1, dtype=np.float32)
    b1 = np.asarray(b1, dtype=np.float32)
    W2 = np.asarray(W2, dtype=np.float32)
    b2 = np.asarray(b2, dtype=np.float32)
    Wv = np.asarray(Wv, dtype=np.float32)
    Wo = np.asarray(Wo, dtype=np.float32)

    xt = _rne12(x.reshape(NTOT, FEAT).T)                      # [1024, 4096]
    w1t = _rne12(np.concatenate([W1.T, W1.T], axis=0))        # [128, 64]
    w2t = _rne12(np.concatenate([W2.T, b2.reshape(1, N_SEQ)], axis=0))  # [65, 2048]
    wot = _rne12(
        Wo.T.reshape(NC, 128, FEAT).transpose(1, 0, 2).reshape(128, NC * FEAT)
    )
    b1c = np.ascontiguousarray(b1.reshape(D, 1))
    sel_h = np.zeros((2, 128), dtype=np.float32)
    sel_h[0, :D] = 1.0
    sel_h[1, D:] = 1.0

    in_maps = []
    for c in range(NC):
        wv_c = _rne12(
            Wv[c * 128 : (c + 1) * 128, :].T
            .reshape(8, 128, 128).transpose(1, 0, 2).reshape(128, FEAT)
        )
        in_maps.append(
            {
                "xt": xt,
                "xc": np.ascontiguousarray(xt[c * 128 : (c + 1) * 128, :]),
                "wv": wv_c,
                "w1t": w1t,
                "b1": b1c,
                "w2t": w2t,
                "wot": wot,
                "sel": sel_h,
            }
        )

    import os
    nc = _get_program(
        int(os.environ.get("KERNEL_REPS", "1")), os.environ.get("KERNEL_PHASES", "A")
    )
    res = run_bass_kernel_spmd(
        nc, in_maps, list(range(NC)), **(_run_kwargs or {})
    )
    out = np.concatenate([res.results[c]["out"] for c in range(NC)], axis=0)
    if _run_kwargs:
        kernel.last_results = res
    return out.reshape(B, N_SEQ, FEAT)

